# revision 15
# baseline (speedup 1.0000x reference)
"""Trainium2 Bass kernel for nn_Critic (LSTM critic, B=512, T=100).

Strategy: data-parallel over batch (64 rows/core x 8 cores).
  Phase A: precompute xz_t = x_t @ Wk + bl for all 2T-1 steps as one big
           batched matmul over (t, b) row tiles; spill to HBM (f32r).
  Phase B: sequential LSTM scan; per step z = xz_t + h @ Wrk computed with
           2-way column-tiled matmuls (batch 64 packed into both halves of
           the 128-wide PE array), gates in a block-interleaved layout
           [i|f|o|g per 128-unit group], elementwise on ACT/DVE, then
           PE-transpose of h back to feature-major for the next step.
All matmuls run in float32r (1 cycle/row at N>=512, ~1e-4 rel precision).
"""
import numpy as np

import concourse.bass as bass
import concourse.mybir as mybir
import concourse.tile as tile
from concourse import bacc
from concourse.bass_utils import run_bass_kernel_spmd
from concourse.masks import make_identity

F32 = mybir.dt.float32
F32R = mybir.dt.float32r
BF16 = mybir.dt.bfloat16
AF = mybir.ActivationFunctionType
ALU = mybir.AluOpType

NCORES = 8
BL = 64  # batch rows per core

_prog_cache = {}


def _elu(nc, pool, out_ap, psum_ap, bias_ap, P, R):
    """out = elu(psum + bias) = relu(x) + min(exp(x), 1) - 1."""
    ee = pool.tile([128, 128], F32, tag="elu_e")
    rr = pool.tile([128, 128], F32, tag="elu_r")
    nc.scalar.activation(ee[:P, :R], psum_ap, AF.Exp, bias=bias_ap)
    nc.scalar.activation(rr[:P, :R], psum_ap, AF.Relu, bias=bias_ap)
    nc.vector.scalar_tensor_tensor(
        ee[:P, :R], ee[:P, :R], 1.0, rr[:P, :R], ALU.min, ALU.add
    )
    nc.vector.tensor_scalar_add(out_ap, ee[:P, :R], -1.0)


def _build(T):
    TT = 2 * T - 1
    NROWT = (TT * BL + 127) // 128  # == T (last tile has 64 rows)

    nc = bacc.Bacc("TRN2", target_bir_lowering=False, num_devices=NCORES)

    # ---- I/O ----
    d_mot = nc.dram_tensor("mot", [BL, 64], F32, kind="ExternalInput")
    d_rob = nc.dram_tensor("rob", [BL, 128], F32, kind="ExternalInput")
    d_s1 = nc.dram_tensor("s1", [TT * BL, 128], F32, kind="ExternalInput")
    d_so = nc.dram_tensor("so", [TT * BL, 128], F32, kind="ExternalInput")
    d_Wm = nc.dram_tensor("Wm", [64, 256], F32, kind="ExternalInput")
    d_bm = nc.dram_tensor("bm", [256, 1], F32, kind="ExternalInput")
    d_Wr = nc.dram_tensor("Wr", [128, 256], F32, kind="ExternalInput")
    d_br = nc.dram_tensor("br", [256, 1], F32, kind="ExternalInput")
    d_Wc = nc.dram_tensor("Wc", [512, 512], F32, kind="ExternalInput")
    d_bc = nc.dram_tensor("bc", [512, 1], F32, kind="ExternalInput")
    d_Wor = nc.dram_tensor("Wor", [128, 256], F32, kind="ExternalInput")
    d_bor = nc.dram_tensor("bor", [256, 1], F32, kind="ExternalInput")
    d_Woi = nc.dram_tensor("Woi", [128, 256], F32, kind="ExternalInput")
    d_boi = nc.dram_tensor("boi", [256, 1], F32, kind="ExternalInput")
    d_Wk = nc.dram_tensor("Wk", [640, 2048], F32, kind="ExternalInput")
    d_Wrk = nc.dram_tensor("Wrk", [512, 2048], F32, kind="ExternalInput")
    d_bl = nc.dram_tensor("bl", [1, 2048], F32, kind="ExternalInput")
    d_Wo = nc.dram_tensor("Wo", [128, 4], F32, kind="ExternalInput")
    d_bo = nc.dram_tensor("bo", [64, 1], F32, kind="ExternalInput")
    d_out = nc.dram_tensor("out", [BL, 1], F32, kind="ExternalOutput")

    with tile.TileContext(nc) as tc:
        with tc.tile_pool(name="consts", bufs=1) as consts, \
             tc.tile_pool(name="wstage", bufs=2) as wstage, \
             tc.tile_pool(name="state", bufs=2) as state, \
             tc.tile_pool(name="dram", bufs=1, space="DRAM") as dpool:

            # ---- identities / ones ----
            identF = consts.tile([128, 128], F32, tag="identF")
            make_identity(nc, identF[:])
            identB = consts.tile([64, 64], BF16, tag="identB")
            nc.vector.tensor_copy(identB[:], identF[0:64, 0:64])
            onesR = consts.tile([1, 128], F32R, tag="onesR")
            onesF = consts.tile([1, 128], F32, tag="onesF")
            nc.gpsimd.memset(onesF[:], 1.0)
            nc.vector.tensor_copy(onesR[:], onesF[:])

            # ---- weights -> f32r ----
            def load_r(dram_ap, shape, tag, kslices=None, dt=F32R):
                t = consts.tile(shape, dt, tag=tag)
                if kslices is None:
                    st = wstage.tile(shape, F32, tag="wtmp_small")
                    nc.sync.dma_start(st[:], dram_ap)
                    nc.vector.tensor_copy(t[:], st[:])
                else:
                    # shape = [128, nk, ncols]; dram [nk*128, ncols]
                    nk, ncols = shape[1], shape[2]
                    for k in range(nk):
                        st = wstage.tile([128, 2048], F32, tag="wtmp")
                        nc.sync.dma_start(
                            st[:, 0:ncols], dram_ap[k * 128:(k + 1) * 128, :]
                        )
                        nc.vector.tensor_copy(t[:, k, :], st[:, 0:ncols])
                return t

            WkR = load_r(d_Wk, [128, 5, 2048], "Wk", kslices=True)
            WrkB = load_r(d_Wrk, [128, 4, 2048], "Wrk", kslices=True, dt=BF16)
            WcR = load_r(d_Wc, [128, 4, 512], "Wc", kslices=True)
            WmR = load_r(d_Wm[:], [64, 256], "Wm")
            WrR = load_r(d_Wr[:], [128, 256], "Wr")
            WorR = load_r(d_Wor[:], [128, 256], "Wor")
            WoiR = load_r(d_Woi[:], [128, 256], "Woi")
            WoB = load_r(d_Wo[:], [128, 4], "Wo", dt=BF16)
            blR = load_r(d_bl[:], [1, 2048], "bl")

            # ---- per-partition biases (f32) ----
            def load_bias(dram, n_tiles, tag):
                t = consts.tile([128, n_tiles], F32, tag=tag)
                for m in range(n_tiles):
                    nc.sync.dma_start(t[:, m:m + 1], dram[m * 128:(m + 1) * 128, :])
                return t

            bm_t = load_bias(d_bm, 2, "bm")
            br_t = load_bias(d_br, 2, "br")
            bc_t = load_bias(d_bc, 4, "bc")
            bor_t = load_bias(d_bor, 2, "bor")
            boi_t = load_bias(d_boi, 2, "boi")
            bo_t = consts.tile([64, 1], F32, tag="bo")
            nc.sync.dma_start(bo_t[:], d_bo[:])

            xz_dram = dpool.tile([TT * BL, 2048], BF16, tag="xz")

            # ================= preamble =================
            with tc.tile_pool(name="pa", bufs=2) as pa, \
                 tc.tile_pool(name="pa_ps", bufs=2, space="PSUM") as pa_ps, \
                 tc.tile_pool(name="pa_tp", bufs=2, space="PSUM") as pa_tp, \
                 tc.tile_pool(name="pa_z", bufs=2, space="PSUM") as pa_z:

                s_mot = pa.tile([64, 64], F32, tag="s_mot")
                nc.sync.dma_start(s_mot[:], d_mot[:])
                s_rob = pa.tile([64, 128], F32, tag="s_rob")
                nc.sync.dma_start(s_rob[:], d_rob[:])

                tp0 = pa_tp.tile([128, 256], F32, tag="tp")
                nc.tensor.transpose(tp0[0:64, 0:64], s_mot[:], identF[0:64, 0:64])
                nc.tensor.transpose(tp0[:, 64:128], s_rob[:], identF[0:64, 0:64])
                motT = pa.tile([64, 64], F32R, tag="motT")
                nc.vector.tensor_copy(motT[:], tp0[0:64, 0:64])
                robT = pa.tile([128, 64], F32R, tag="robT")
                nc.vector.tensor_copy(robT[:], tp0[:, 64:128])

                # ms / rs  (feature-major [256, 64] as [128, 2*64])
                msT = pa.tile([128, 128], F32R, tag="msT")
                rsT = pa.tile([128, 128], F32R, tag="rsT")
                ps_m = pa_ps.tile([128, 128], F32, tag="small")
                for m in range(2):
                    nc.tensor.matmul(ps_m[:, m * 64:(m + 1) * 64],
                                     WmR[0:64, m * 128:(m + 1) * 128], motT[:],
                                     start=True, stop=True)
                for m in range(2):
                    _elu(nc, pa, msT[:, m * 64:(m + 1) * 64],
                         ps_m[:, m * 64:(m + 1) * 64], bm_t[:, m:m + 1], 128, 64)
                ps_r = pa_ps.tile([128, 128], F32, tag="small")
                for m in range(2):
                    nc.tensor.matmul(ps_r[:, m * 64:(m + 1) * 64],
                                     WrR[:, m * 128:(m + 1) * 128], robT[:],
                                     start=True, stop=True)
                for m in range(2):
                    _elu(nc, pa, rsT[:, m * 64:(m + 1) * 64],
                         ps_r[:, m * 64:(m + 1) * 64], br_t[:, m:m + 1], 128, 64)

                # state = elu([ms, rs] @ Wc + bc) -> st01/st23 (hT layout) + c0
                st01 = state.tile([128, 128], BF16, tag="hT01")
                st23 = state.tile([128, 128], BF16, tag="hT23")
                stF = pa.tile([128, 256], F32, tag="stF")
                ps_c = pa_ps.tile([128, 256], F32, tag="small")
                for G in range(4):
                    reg = ps_c[:, G * 64:(G + 1) * 64]
                    for k in range(4):
                        rhs = msT[:, (k % 2) * 64:(k % 2) * 64 + 64] if k < 2 \
                            else rsT[:, (k % 2) * 64:(k % 2) * 64 + 64]
                        nc.tensor.matmul(reg, WcR[:, k, G * 128:(G + 1) * 128],
                                         rhs, start=(k == 0), stop=(k == 3))
                for G in range(4):
                    _elu(nc, pa, stF[:, G * 64:(G + 1) * 64],
                         ps_c[:, G * 64:(G + 1) * 64], bc_t[:, G:G + 1], 128, 64)
                nc.vector.tensor_copy(st01[:], stF[:, 0:128])
                nc.vector.tensor_copy(st23[:], stF[:, 128:256])

                c0 = state.tile([128, 256], F32, tag="c")
                tp1 = pa_tp.tile([128, 256], F32, tag="tp")
                nc.tensor.transpose(tp1[:, 0:128], stF[:, 0:128], identF[:])
                nc.tensor.transpose(tp1[:, 128:256], stF[:, 128:256], identF[:])
                nc.vector.tensor_copy(c0[:, 0:128], tp1[:, 0:128])
                nc.vector.tensor_copy(c0[:, 128:256], tp1[:, 128:256])

                # ================= phase A =================
                for R in range(NROWT):
                    rows = 128 if R < NROWT - 1 else TT * BL - 128 * (NROWT - 1)
                    base = R * 128

                    s1t = pa.tile([128, 128], F32, tag="s1t")
                    nc.sync.dma_start(s1t[0:rows, :], d_s1[base:base + rows, :])
                    sot = pa.tile([128, 128], F32, tag="sot")
                    nc.sync.dma_start(sot[0:rows, :], d_so[base:base + rows, :])

                    tpa = pa_tp.tile([128, 256], F32, tag="tp")
                    nc.tensor.transpose(tpa[:, 0:rows], sot[0:rows, :],
                                        identF[0:rows, 0:rows])
                    nc.tensor.transpose(tpa[:, 128:128 + rows], s1t[0:rows, :],
                                        identF[0:rows, 0:rows])
                    oscT = pa.tile([128, 128], F32R, tag="oscT")
                    nc.vector.tensor_copy(oscT[:, 0:rows], tpa[:, 0:rows])
                    inp1T = pa.tile([128, 128], F32R, tag="inp1T")
                    nc.vector.tensor_copy(inp1T[:, 0:rows], tpa[:, 128:128 + rows])

                    # inp2 = elu(osc_half @ Wor + bor)
                    i2T = pa.tile([128, 256], F32R, tag="i2T")
                    ps2 = pa_ps.tile([128, 256], F32, tag="small")
                    for m in range(2):
                        nc.tensor.matmul(ps2[:, m * 128:m * 128 + rows],
                                         WorR[:, m * 128:(m + 1) * 128],
                                         oscT[:, 0:rows], start=True, stop=True)
                    for m in range(2):
                        _elu(nc, pa, i2T[:, m * 128:m * 128 + rows],
                             ps2[:, m * 128:m * 128 + rows], bor_t[:, m:m + 1],
                             128, rows)

                    # inp3 = elu(inp2[:, 128:256] @ Woi + boi)
                    i3T = pa.tile([128, 256], F32R, tag="i3T")
                    ps3 = pa_ps.tile([128, 256], F32, tag="small")
                    for m in range(2):
                        nc.tensor.matmul(ps3[:, m * 128:m * 128 + rows],
                                         WoiR[:, m * 128:(m + 1) * 128],
                                         i2T[:, 128:128 + rows], start=True, stop=True)
                    for m in range(2):
                        _elu(nc, pa, i3T[:, m * 128:m * 128 + rows],
                             ps3[:, m * 128:m * 128 + rows], boi_t[:, m:m + 1],
                             128, rows)

                    # xz = x @ Wk + bl   (x = [inp1, inp2, inp3], feature-major k-tiles)
                    lhs_list = [inp1T[:, 0:rows], i2T[:, 0:rows], i2T[:, 128:128 + rows],
                                i3T[:, 0:rows], i3T[:, 128:128 + rows]]
                    xz_sb = pa.tile([128, 2048], BF16, tag="xz_sb")
                    for h in range(2):
                        psz = pa_z.tile([128, 1024], F32, tag="z")
                        for nsub in range(2):
                            n = 2 * h + nsub
                            reg = psz[0:rows, nsub * 512:(nsub + 1) * 512]
                            for k in range(5):
                                nc.tensor.matmul(reg, lhs_list[k],
                                                 WkR[:, k, n * 512:(n + 1) * 512],
                                                 start=(k == 0), stop=False)
                            nc.tensor.matmul(reg, onesR[0:1, 0:rows],
                                             blR[0:1, n * 512:(n + 1) * 512],
                                             start=False, stop=True)
                        nc.vector.tensor_copy(xz_sb[0:rows, h * 1024:(h + 1) * 1024],
                                              psz[0:rows, :])
                    nc.sync.dma_start(xz_dram[base:base + rows, :], xz_sb[0:rows, :])

            # ================= phase B =================
            with tc.tile_pool(name="pb", bufs=2) as pb, \
                 tc.tile_pool(name="xzp", bufs=4) as xzp, \
                 tc.tile_pool(name="pb_z", bufs=2, space="PSUM") as pb_z, \
                 tc.tile_pool(name="pb_tp", bufs=2, space="PSUM") as pb_tp:

                hT01, hT23, c_prev = st01, st23, c0
                for t in range(TT):
                    xzt = xzp.tile([64, 2048], BF16, tag="xzt")
                    nc.sync.dma_start(xzt[:], xz_dram[t * BL:(t + 1) * BL, :])

                    zp = pb_z.tile([128, 1024], F32, tag="z")
                    for b in range(2):
                        lo = zp[0:64, b * 512:(b + 1) * 512]
                        hi = zp[64:128, b * 512:(b + 1) * 512]
                        # xz inject first (no dependency on h -> overlaps prev tail)
                        nc.tensor.matmul(lo, identB[:],
                                         xzt[:, (2 * b) * 512:(2 * b + 1) * 512],
                                         start=True, stop=False, tile_position=(0, 0))
                        nc.tensor.matmul(hi, identB[:],
                                         xzt[:, (2 * b + 1) * 512:(2 * b + 2) * 512],
                                         start=True, stop=False, tile_position=(0, 64))
                        for G in range(4):
                            lhsT = (hT01 if G < 2 else hT23)[:, (G % 2) * 64:(G % 2) * 64 + 64]
                            nc.tensor.matmul(lo, lhsT,
                                             WrkB[:, G, (2 * b) * 512:(2 * b + 1) * 512],
                                             start=False, stop=(G == 3),
                                             tile_position=(0, 0))
                            nc.tensor.matmul(hi, lhsT,
                                             WrkB[:, G, (2 * b + 1) * 512:(2 * b + 2) * 512],
                                             start=False, stop=(G == 3),
                                             tile_position=(0, 64))

                    c_new = state.tile([128, 256], F32, tag="c")
                    hT01_n = state.tile([128, 128], BF16, tag="hT01")
                    hT23_n = state.tile([128, 128], BF16, tag="hT23")
                    tpb = pb_tp.tile([128, 256], F32, tag="tp")
                    for b in range(2):
                        sg = pb.tile([128, 384], F32, tag="sg")
                        nc.scalar.activation(sg[:], zp[:, b * 512:b * 512 + 384],
                                             AF.Sigmoid)
                        tg = pb.tile([128, 128], F32, tag="tg")
                        nc.scalar.activation(tg[:], zp[:, b * 512 + 384:b * 512 + 512],
                                             AF.Tanh)
                        t1 = pb.tile([128, 128], F32, tag="t1")
                        nc.vector.tensor_mul(t1[:], sg[:, 0:128], tg[:])
                        t2 = pb.tile([128, 128], F32, tag="t2")
                        nc.vector.tensor_mul(t2[:], sg[:, 128:256],
                                             c_prev[:, b * 128:(b + 1) * 128])
                        cs = c_new[:, b * 128:(b + 1) * 128]
                        nc.vector.tensor_add(cs, t1[:], t2[:])
                        tcn = pb.tile([128, 128], F32, tag="tc")
                        nc.scalar.activation(tcn[:], cs, AF.Tanh)
                        hb = pb.tile([128, 128], F32, tag="hb")
                        nc.vector.tensor_mul(hb[:], sg[:, 256:384], tcn[:])
                        nc.tensor.transpose(tpb[:, b * 128:(b + 1) * 128], hb[:],
                                            identF[:])
                        dst = hT01_n if b == 0 else hT23_n
                        nc.vector.tensor_copy(dst[:], tpb[:, b * 128:(b + 1) * 128])
                    hT01, hT23, c_prev = hT01_n, hT23_n, c_new

                # ---- output: elu(h @ Wo + bo) ----
                ps_o = pb_z.tile([64, 512], F32, tag="z")
                for G in range(4):
                    lhsT = (hT01 if G < 2 else hT23)[:, (G % 2) * 64:(G % 2) * 64 + 64]
                    nc.tensor.matmul(ps_o[:, 0:1], lhsT, WoB[:, G:G + 1],
                                     start=(G == 0), stop=(G == 3))
                out_sb = pb.tile([64, 1], F32, tag="out_sb")
                ee = pb.tile([64, 1], F32, tag="oee")
                rr = pb.tile([64, 1], F32, tag="orr")
                nc.scalar.activation(ee[:], ps_o[:, 0:1], AF.Exp, bias=bo_t[:])
                nc.scalar.activation(rr[:], ps_o[:, 0:1], AF.Relu, bias=bo_t[:])
                nc.vector.scalar_tensor_tensor(ee[:], ee[:], 1.0, rr[:],
                                               ALU.min, ALU.add)
                nc.vector.tensor_scalar_add(out_sb[:], ee[:], -1.0)
                nc.sync.dma_start(d_out[:], out_sb[:])

    nc.compile()
    return nc


def _build_baseline():
    """Trivial program for dispatch-overhead calibration."""
    nc = bacc.Bacc("TRN2", target_bir_lowering=False, num_devices=NCORES)
    x = nc.dram_tensor("x", [16, 16], F32, kind="ExternalInput")
    y = nc.dram_tensor("y", [16, 16], F32, kind="ExternalOutput")
    with tile.TileContext(nc) as tc:
        with tc.tile_pool(name="sbuf", bufs=1) as pool:
            t = pool.tile([16, 16], F32)
            nc.sync.dma_start(t[:], x[:])
            nc.sync.dma_start(y[:], t[:])
    nc.compile()
    return nc


def _gate_perm():
    """Column permutation: [i|f|g|o] blocks of 512 -> per-128-unit-group [i f o g]."""
    perm = []
    for G in range(4):
        for gate in (0, 1, 3, 2):  # i, f, o, g
            perm.extend(range(gate * 512 + G * 128, gate * 512 + (G + 1) * 128))
    return np.array(perm)


def _prepare(inputs):
    motion_state = np.asarray(inputs["motion_state"], np.float32)
    robot_state = np.asarray(inputs["robot_state"], np.float32)
    action = np.asarray(inputs["action"], np.float32)
    osc = np.asarray(inputs["osc"], np.float32)
    history = np.asarray(inputs["history"], np.float32)
    history_osc = np.asarray(inputs["history_osc"], np.float32)

    B, T = action.shape[0], action.shape[1]
    assert B == NCORES * BL
    TT = 2 * T - 1

    perm = _gate_perm()
    Wk_p = np.ascontiguousarray(np.asarray(inputs["Wk"], np.float32)[:, perm])
    Wrk_p = np.ascontiguousarray(np.asarray(inputs["Wrk"], np.float32)[:, perm])
    bl_p = np.ascontiguousarray(
        np.asarray(inputs["bl"], np.float32)[perm].reshape(1, 2048))
    Wo = np.asarray(inputs["Wo"], np.float32)  # [512, 1]
    Wo_t = np.ascontiguousarray(Wo.reshape(4, 128).T)  # [128, 4]
    bo = np.asarray(inputs["bo"], np.float32)
    bo_t = np.full((64, 1), float(bo[0]), np.float32)

    # streams: warmup over last T-1 history frames, then action/osc
    seq1 = np.concatenate([history[:, 1:], action], axis=1)          # [B, TT, 128]
    seqo = np.concatenate([history_osc[:, 1:, 0:128], osc[:, :, 0:128]], axis=1)
    seq1_tm = np.swapaxes(seq1, 0, 1)                                # [TT, B, 128]
    seqo_tm = np.swapaxes(seqo, 0, 1)

    shared = {
        "Wm": np.asarray(inputs["Wm"], np.float32),
        "bm": np.asarray(inputs["bm"], np.float32).reshape(256, 1),
        "Wr": np.asarray(inputs["Wr"], np.float32),
        "br": np.asarray(inputs["br"], np.float32).reshape(256, 1),
        "Wc": np.asarray(inputs["Wc"], np.float32),
        "bc": np.asarray(inputs["bc"], np.float32).reshape(512, 1),
        "Wor": np.asarray(inputs["Wor"], np.float32),
        "bor": np.asarray(inputs["bor"], np.float32).reshape(256, 1),
        "Woi": np.asarray(inputs["Woi"], np.float32),
        "boi": np.asarray(inputs["boi"], np.float32).reshape(256, 1),
        "Wk": Wk_p, "Wrk": Wrk_p, "bl": bl_p, "Wo": Wo_t, "bo": bo_t,
    }

    in_maps = []
    for c in range(NCORES):
        sl = slice(c * BL, (c + 1) * BL)
        m = dict(shared)
        m["mot"] = np.ascontiguousarray(motion_state[sl])
        m["rob"] = np.ascontiguousarray(robot_state[sl])
        m["s1"] = np.ascontiguousarray(seq1_tm[:, sl]).reshape(TT * BL, 128)
        m["so"] = np.ascontiguousarray(seqo_tm[:, sl]).reshape(TT * BL, 128)
        in_maps.append(m)

    return in_maps, T


def kernel(**inputs):
    in_maps, T = _prepare(inputs)
    if T not in _prog_cache:
        _prog_cache[T] = _build(T)
    nc = _prog_cache[T]

    res = run_bass_kernel_spmd(nc, in_maps, core_ids=list(range(NCORES)))
    out = np.concatenate([res.results[c]["out"] for c in range(NCORES)], axis=0)
    return out.astype(np.float32)


# revision 18
# speedup vs baseline: 1.4872x; 1.4872x over previous
"""Trainium2 Bass kernel for nn_Critic (LSTM critic, B=512, T=100).

Strategy: data-parallel over batch (64 rows/core x 8 cores).
  Phase A: precompute xz_t = x_t @ Wk + bl for all 2T-1 steps as one big
           batched matmul over (t, b) row tiles; spill to HBM (f32r).
  Phase B: sequential LSTM scan; per step z = xz_t + h @ Wrk computed with
           2-way column-tiled matmuls (batch 64 packed into both halves of
           the 128-wide PE array), gates in a block-interleaved layout
           [i|f|o|g per 128-unit group], elementwise on ACT/DVE, then
           PE-transpose of h back to feature-major for the next step.
All matmuls run in float32r (1 cycle/row at N>=512, ~1e-4 rel precision).
"""
import numpy as np

import concourse.bass as bass
import concourse.mybir as mybir
import concourse.tile as tile
from concourse import bacc
from concourse.bass_utils import run_bass_kernel_spmd
from concourse.masks import make_identity

F32 = mybir.dt.float32
F32R = mybir.dt.float32r
BF16 = mybir.dt.bfloat16
AF = mybir.ActivationFunctionType
ALU = mybir.AluOpType

NCORES = 8
BL = 64  # batch rows per core

_prog_cache = {}


def _elu(nc, pool, out_ap, psum_ap, bias_ap, P, R):
    """out = elu(psum + bias) = relu(x) + min(exp(x), 1) - 1."""
    ee = pool.tile([128, 128], F32, tag="elu_e")
    rr = pool.tile([128, 128], F32, tag="elu_r")
    nc.scalar.activation(ee[:P, :R], psum_ap, AF.Exp, bias=bias_ap)
    nc.scalar.activation(rr[:P, :R], psum_ap, AF.Relu, bias=bias_ap)
    nc.vector.scalar_tensor_tensor(
        ee[:P, :R], ee[:P, :R], 1.0, rr[:P, :R], ALU.min, ALU.add
    )
    nc.vector.tensor_scalar_add(out_ap, ee[:P, :R], -1.0)


def _build(T):
    TT = 2 * T - 1
    NROWT = (TT * BL + 127) // 128  # == T (last tile has 64 rows)

    nc = bacc.Bacc("TRN2", target_bir_lowering=False, num_devices=NCORES)

    # ---- I/O ----
    d_mot = nc.dram_tensor("mot", [BL, 64], F32, kind="ExternalInput")
    d_rob = nc.dram_tensor("rob", [BL, 128], F32, kind="ExternalInput")
    d_s1 = nc.dram_tensor("s1", [TT * BL, 128], F32, kind="ExternalInput")
    d_so = nc.dram_tensor("so", [TT * BL, 128], F32, kind="ExternalInput")
    d_Wm = nc.dram_tensor("Wm", [64, 256], F32, kind="ExternalInput")
    d_bm = nc.dram_tensor("bm", [256, 1], F32, kind="ExternalInput")
    d_Wr = nc.dram_tensor("Wr", [128, 256], F32, kind="ExternalInput")
    d_br = nc.dram_tensor("br", [256, 1], F32, kind="ExternalInput")
    d_Wc = nc.dram_tensor("Wc", [512, 512], F32, kind="ExternalInput")
    d_bc = nc.dram_tensor("bc", [512, 1], F32, kind="ExternalInput")
    d_Wor = nc.dram_tensor("Wor", [128, 256], F32, kind="ExternalInput")
    d_bor = nc.dram_tensor("bor", [256, 1], F32, kind="ExternalInput")
    d_Woi = nc.dram_tensor("Woi", [128, 256], F32, kind="ExternalInput")
    d_boi = nc.dram_tensor("boi", [256, 1], F32, kind="ExternalInput")
    d_Wk = nc.dram_tensor("Wk", [640, 2048], F32, kind="ExternalInput")
    d_Wrk = nc.dram_tensor("Wrk", [512, 2048], F32, kind="ExternalInput")
    d_bl = nc.dram_tensor("bl", [1, 2048], F32, kind="ExternalInput")
    d_Wo = nc.dram_tensor("Wo", [128, 4], F32, kind="ExternalInput")
    d_bo = nc.dram_tensor("bo", [64, 1], F32, kind="ExternalInput")
    d_out = nc.dram_tensor("out", [BL, 1], F32, kind="ExternalOutput")

    with tile.TileContext(nc) as tc:
        with tc.tile_pool(name="consts", bufs=1) as consts, \
             tc.tile_pool(name="wstage", bufs=2) as wstage, \
             tc.tile_pool(name="state", bufs=2) as state, \
             tc.tile_pool(name="dram", bufs=1, space="DRAM") as dpool:

            # ---- identities / ones ----
            identF = consts.tile([128, 128], F32, tag="identF")
            make_identity(nc, identF[:])
            identB = consts.tile([64, 64], BF16, tag="identB")
            nc.vector.tensor_copy(identB[:], identF[0:64, 0:64])
            onesR = consts.tile([1, 128], F32R, tag="onesR")
            onesF = consts.tile([1, 128], F32, tag="onesF")
            nc.gpsimd.memset(onesF[:], 1.0)
            nc.vector.tensor_copy(onesR[:], onesF[:])

            # ---- weights -> f32r ----
            def load_r(dram_ap, shape, tag, kslices=None, dt=F32R):
                t = consts.tile(shape, dt, tag=tag)
                if kslices is None:
                    st = wstage.tile(shape, F32, tag="wtmp_small")
                    nc.sync.dma_start(st[:], dram_ap)
                    nc.vector.tensor_copy(t[:], st[:])
                else:
                    # shape = [128, nk, ncols]; dram [nk*128, ncols]
                    nk, ncols = shape[1], shape[2]
                    for k in range(nk):
                        st = wstage.tile([128, 2048], F32, tag="wtmp")
                        nc.sync.dma_start(
                            st[:, 0:ncols], dram_ap[k * 128:(k + 1) * 128, :]
                        )
                        nc.vector.tensor_copy(t[:, k, :], st[:, 0:ncols])
                return t

            WkR = load_r(d_Wk, [128, 5, 2048], "Wk", kslices=True)
            WrkB = load_r(d_Wrk, [128, 4, 2048], "Wrk", kslices=True, dt=BF16)
            WcR = load_r(d_Wc, [128, 4, 512], "Wc", kslices=True)
            WmR = load_r(d_Wm[:], [64, 256], "Wm")
            WrR = load_r(d_Wr[:], [128, 256], "Wr")
            WorR = load_r(d_Wor[:], [128, 256], "Wor")
            WoiR = load_r(d_Woi[:], [128, 256], "Woi")
            WoB = load_r(d_Wo[:], [128, 4], "Wo", dt=BF16)
            blR = load_r(d_bl[:], [1, 2048], "bl")

            # ---- per-partition biases (f32) ----
            def load_bias(dram, n_tiles, tag):
                t = consts.tile([128, n_tiles], F32, tag=tag)
                for m in range(n_tiles):
                    nc.sync.dma_start(t[:, m:m + 1], dram[m * 128:(m + 1) * 128, :])
                return t

            bm_t = load_bias(d_bm, 2, "bm")
            br_t = load_bias(d_br, 2, "br")
            bc_t = load_bias(d_bc, 4, "bc")
            bor_t = load_bias(d_bor, 2, "bor")
            boi_t = load_bias(d_boi, 2, "boi")
            bo_t = consts.tile([64, 1], F32, tag="bo")
            nc.sync.dma_start(bo_t[:], d_bo[:])

            xz_dram = dpool.tile([TT * BL, 2048], BF16, tag="xz")

            # ================= preamble =================
            with tc.tile_pool(name="pa", bufs=2) as pa, \
                 tc.tile_pool(name="pa_ps", bufs=2, space="PSUM") as pa_ps, \
                 tc.tile_pool(name="pa_tp", bufs=2, space="PSUM") as pa_tp, \
                 tc.tile_pool(name="pa_z", bufs=2, space="PSUM") as pa_z:

                s_mot = pa.tile([64, 64], F32, tag="s_mot")
                nc.sync.dma_start(s_mot[:], d_mot[:])
                s_rob = pa.tile([64, 128], F32, tag="s_rob")
                nc.sync.dma_start(s_rob[:], d_rob[:])

                tp0 = pa_tp.tile([128, 256], F32, tag="tp")
                nc.tensor.transpose(tp0[0:64, 0:64], s_mot[:], identF[0:64, 0:64])
                nc.tensor.transpose(tp0[:, 64:128], s_rob[:], identF[0:64, 0:64])
                motT = pa.tile([64, 64], F32R, tag="motT")
                nc.vector.tensor_copy(motT[:], tp0[0:64, 0:64])
                robT = pa.tile([128, 64], F32R, tag="robT")
                nc.vector.tensor_copy(robT[:], tp0[:, 64:128])

                # ms / rs  (feature-major [256, 64] as [128, 2*64])
                msT = pa.tile([128, 128], F32R, tag="msT")
                rsT = pa.tile([128, 128], F32R, tag="rsT")
                ps_m = pa_ps.tile([128, 128], F32, tag="small")
                for m in range(2):
                    nc.tensor.matmul(ps_m[:, m * 64:(m + 1) * 64],
                                     WmR[0:64, m * 128:(m + 1) * 128], motT[:],
                                     start=True, stop=True)
                for m in range(2):
                    _elu(nc, pa, msT[:, m * 64:(m + 1) * 64],
                         ps_m[:, m * 64:(m + 1) * 64], bm_t[:, m:m + 1], 128, 64)
                ps_r = pa_ps.tile([128, 128], F32, tag="small")
                for m in range(2):
                    nc.tensor.matmul(ps_r[:, m * 64:(m + 1) * 64],
                                     WrR[:, m * 128:(m + 1) * 128], robT[:],
                                     start=True, stop=True)
                for m in range(2):
                    _elu(nc, pa, rsT[:, m * 64:(m + 1) * 64],
                         ps_r[:, m * 64:(m + 1) * 64], br_t[:, m:m + 1], 128, 64)

                # state = elu([ms, rs] @ Wc + bc) -> st01/st23 (hT layout) + c0
                st01 = state.tile([128, 128], BF16, tag="hT01")
                st23 = state.tile([128, 128], BF16, tag="hT23")
                stF = pa.tile([128, 256], F32, tag="stF")
                ps_c = pa_ps.tile([128, 256], F32, tag="small")
                for G in range(4):
                    reg = ps_c[:, G * 64:(G + 1) * 64]
                    for k in range(4):
                        rhs = msT[:, (k % 2) * 64:(k % 2) * 64 + 64] if k < 2 \
                            else rsT[:, (k % 2) * 64:(k % 2) * 64 + 64]
                        nc.tensor.matmul(reg, WcR[:, k, G * 128:(G + 1) * 128],
                                         rhs, start=(k == 0), stop=(k == 3))
                for G in range(4):
                    _elu(nc, pa, stF[:, G * 64:(G + 1) * 64],
                         ps_c[:, G * 64:(G + 1) * 64], bc_t[:, G:G + 1], 128, 64)
                nc.vector.tensor_copy(st01[:], stF[:, 0:128])
                nc.vector.tensor_copy(st23[:], stF[:, 128:256])

                c0 = state.tile([128, 256], F32, tag="c")
                tp1 = pa_tp.tile([128, 256], F32, tag="tp")
                nc.tensor.transpose(tp1[:, 0:128], stF[:, 0:128], identF[:])
                nc.tensor.transpose(tp1[:, 128:256], stF[:, 128:256], identF[:])
                nc.vector.tensor_copy(c0[:, 0:128], tp1[:, 0:128])
                nc.vector.tensor_copy(c0[:, 128:256], tp1[:, 128:256])

                # ================= phase A (software-pipelined) =================
                def pa_stage1(R, rows):
                    base = R * 128
                    s1t = pa.tile([128, 128], F32, tag="s1t")
                    nc.sync.dma_start(s1t[0:rows, :], d_s1[base:base + rows, :])
                    sot = pa.tile([128, 128], F32, tag="sot")
                    nc.sync.dma_start(sot[0:rows, :], d_so[base:base + rows, :])

                    tpa = pa_tp.tile([128, 256], F32, tag="tp")
                    nc.tensor.transpose(tpa[:, 0:rows], sot[0:rows, :],
                                        identF[0:rows, 0:rows])
                    nc.tensor.transpose(tpa[:, 128:128 + rows], s1t[0:rows, :],
                                        identF[0:rows, 0:rows])
                    oscT = pa.tile([128, 128], F32R, tag="oscT")
                    nc.vector.tensor_copy(oscT[:, 0:rows], tpa[:, 0:rows])
                    inp1T = pa.tile([128, 128], F32R, tag="inp1T")
                    nc.vector.tensor_copy(inp1T[:, 0:rows], tpa[:, 128:128 + rows])

                    # inp2 = elu(osc_half @ Wor + bor)
                    i2T = pa.tile([128, 256], F32R, tag="i2T")
                    ps2 = pa_ps.tile([128, 256], F32, tag="small")
                    for m in range(2):
                        nc.tensor.matmul(ps2[:, m * 128:m * 128 + rows],
                                         WorR[:, m * 128:(m + 1) * 128],
                                         oscT[:, 0:rows], start=True, stop=True)
                    for m in range(2):
                        _elu(nc, pa, i2T[:, m * 128:m * 128 + rows],
                             ps2[:, m * 128:m * 128 + rows], bor_t[:, m:m + 1],
                             128, rows)
                    return inp1T, i2T

                def pa_stage2(R, rows, i2T):
                    # inp3 = elu(inp2[:, 128:256] @ Woi + boi)
                    i3T = pa.tile([128, 256], F32R, tag="i3T")
                    ps3 = pa_ps.tile([128, 256], F32, tag="small")
                    for m in range(2):
                        nc.tensor.matmul(ps3[:, m * 128:m * 128 + rows],
                                         WoiR[:, m * 128:(m + 1) * 128],
                                         i2T[:, 128:128 + rows], start=True, stop=True)
                    for m in range(2):
                        _elu(nc, pa, i3T[:, m * 128:m * 128 + rows],
                             ps3[:, m * 128:m * 128 + rows], boi_t[:, m:m + 1],
                             128, rows)
                    return i3T

                def pa_stage3(R, rows, inp1T, i2T, i3T):
                    # xz = x @ Wk + bl  (x = [inp1, inp2, inp3], feature-major)
                    base = R * 128
                    lhs_list = [inp1T[:, 0:rows], i2T[:, 0:rows], i2T[:, 128:128 + rows],
                                i3T[:, 0:rows], i3T[:, 128:128 + rows]]
                    xz_sb = pa.tile([128, 2048], BF16, tag="xz_sb")
                    for h in range(2):
                        psz = pa_z.tile([128, 1024], F32, tag="z")
                        for nsub in range(2):
                            n = 2 * h + nsub
                            reg = psz[0:rows, nsub * 512:(nsub + 1) * 512]
                            for k in range(5):
                                nc.tensor.matmul(reg, lhs_list[k],
                                                 WkR[:, k, n * 512:(n + 1) * 512],
                                                 start=(k == 0), stop=False)
                            nc.tensor.matmul(reg, onesR[0:1, 0:rows],
                                             blR[0:1, n * 512:(n + 1) * 512],
                                             start=False, stop=True)
                        nc.vector.tensor_copy(xz_sb[0:rows, h * 1024:(h + 1) * 1024],
                                              psz[0:rows, :])
                    nc.sync.dma_start(xz_dram[base:base + rows, :], xz_sb[0:rows, :])

                prev = None
                for R in range(NROWT):
                    rows = 128 if R < NROWT - 1 else TT * BL - 128 * (NROWT - 1)
                    inp1T, i2T = pa_stage1(R, rows)
                    if prev is not None:
                        pa_stage3(*prev)
                    i3T = pa_stage2(R, rows, i2T)
                    prev = (R, rows, inp1T, i2T, i3T)
                pa_stage3(*prev)

            # ================= phase B =================
            with tc.tile_pool(name="pb", bufs=2) as pb, \
                 tc.tile_pool(name="xzp", bufs=4) as xzp, \
                 tc.tile_pool(name="pb_z", bufs=2, space="PSUM") as pb_z, \
                 tc.tile_pool(name="pb_tp", bufs=2, space="PSUM") as pb_tp:

                def emit_inject(t):
                    # xz load + identity-inject: depends only on DMA, so these
                    # PE slots fill the previous step's gate->h->transpose tail.
                    xzt = xzp.tile([64, 2048], BF16, tag="xzt")
                    nc.sync.dma_start(xzt[:], xz_dram[t * BL:(t + 1) * BL, :])
                    zp = pb_z.tile([128, 1024], F32, tag="z")
                    for b in range(2):
                        nc.tensor.matmul(zp[0:64, b * 512:(b + 1) * 512], identB[:],
                                         xzt[:, (2 * b) * 512:(2 * b + 1) * 512],
                                         start=True, stop=False, tile_position=(0, 0))
                        nc.tensor.matmul(zp[64:128, b * 512:(b + 1) * 512], identB[:],
                                         xzt[:, (2 * b + 1) * 512:(2 * b + 2) * 512],
                                         start=True, stop=False, tile_position=(0, 64))
                    return zp

                hT01, hT23, c_prev = st01, st23, c0
                zp_cur = emit_inject(0)
                for t in range(TT):
                    zp = zp_cur
                    # G0/G1 need only hT01 (ready early); hT23 is consumed
                    # ~1.3us into the step's PE work.
                    for G in range(4):
                        lhsT = (hT01 if G < 2 else hT23)[:, (G % 2) * 64:(G % 2) * 64 + 64]
                        for b in range(2):
                            nc.tensor.matmul(zp[0:64, b * 512:(b + 1) * 512], lhsT,
                                             WrkB[:, G, (2 * b) * 512:(2 * b + 1) * 512],
                                             start=False, stop=(G == 3),
                                             tile_position=(0, 0))
                            nc.tensor.matmul(zp[64:128, b * 512:(b + 1) * 512], lhsT,
                                             WrkB[:, G, (2 * b + 1) * 512:(2 * b + 2) * 512],
                                             start=False, stop=(G == 3),
                                             tile_position=(0, 64))
                    zp_cur = emit_inject(t + 1) if t + 1 < TT else None

                    c_new = state.tile([128, 256], F32, tag="c")
                    hT01_n = state.tile([128, 128], BF16, tag="hT01")
                    hT23_n = state.tile([128, 128], BF16, tag="hT23")
                    tpb = pb_tp.tile([128, 256], F32, tag="tp")
                    for b in range(2):
                        sg = pb.tile([128, 384], F32, tag="sg")
                        nc.scalar.activation(sg[:], zp[:, b * 512:b * 512 + 384],
                                             AF.Sigmoid)
                        tg = pb.tile([128, 128], F32, tag="tg")
                        nc.scalar.activation(tg[:], zp[:, b * 512 + 384:b * 512 + 512],
                                             AF.Tanh)
                        t1 = pb.tile([128, 128], F32, tag="t1")
                        nc.vector.tensor_mul(t1[:], sg[:, 0:128], tg[:])
                        t2 = pb.tile([128, 128], F32, tag="t2")
                        nc.vector.tensor_mul(t2[:], sg[:, 128:256],
                                             c_prev[:, b * 128:(b + 1) * 128])
                        cs = c_new[:, b * 128:(b + 1) * 128]
                        nc.vector.tensor_add(cs, t1[:], t2[:])
                        tcn = pb.tile([128, 128], F32, tag="tc")
                        nc.scalar.activation(tcn[:], cs, AF.Tanh)
                        hb = pb.tile([128, 128], F32, tag="hb")
                        nc.vector.tensor_mul(hb[:], sg[:, 256:384], tcn[:])
                        nc.tensor.transpose(tpb[:, b * 128:(b + 1) * 128], hb[:],
                                            identF[:])
                        dst = hT01_n if b == 0 else hT23_n
                        nc.vector.tensor_copy(dst[:], tpb[:, b * 128:(b + 1) * 128])
                    hT01, hT23, c_prev = hT01_n, hT23_n, c_new

                # ---- output: elu(h @ Wo + bo) ----
                ps_o = pb_z.tile([64, 512], F32, tag="z")
                for G in range(4):
                    lhsT = (hT01 if G < 2 else hT23)[:, (G % 2) * 64:(G % 2) * 64 + 64]
                    nc.tensor.matmul(ps_o[:, 0:1], lhsT, WoB[:, G:G + 1],
                                     start=(G == 0), stop=(G == 3))
                out_sb = pb.tile([64, 1], F32, tag="out_sb")
                ee = pb.tile([64, 1], F32, tag="oee")
                rr = pb.tile([64, 1], F32, tag="orr")
                nc.scalar.activation(ee[:], ps_o[:, 0:1], AF.Exp, bias=bo_t[:])
                nc.scalar.activation(rr[:], ps_o[:, 0:1], AF.Relu, bias=bo_t[:])
                nc.vector.scalar_tensor_tensor(ee[:], ee[:], 1.0, rr[:],
                                               ALU.min, ALU.add)
                nc.vector.tensor_scalar_add(out_sb[:], ee[:], -1.0)
                nc.sync.dma_start(d_out[:], out_sb[:])

    nc.compile()
    return nc


def _build_baseline():
    """Trivial program for dispatch-overhead calibration."""
    nc = bacc.Bacc("TRN2", target_bir_lowering=False, num_devices=NCORES)
    x = nc.dram_tensor("x", [16, 16], F32, kind="ExternalInput")
    y = nc.dram_tensor("y", [16, 16], F32, kind="ExternalOutput")
    with tile.TileContext(nc) as tc:
        with tc.tile_pool(name="sbuf", bufs=1) as pool:
            t = pool.tile([16, 16], F32)
            nc.sync.dma_start(t[:], x[:])
            nc.sync.dma_start(y[:], t[:])
    nc.compile()
    return nc


def _gate_perm():
    """Column permutation: [i|f|g|o] blocks of 512 -> per-128-unit-group [i f o g]."""
    perm = []
    for G in range(4):
        for gate in (0, 1, 3, 2):  # i, f, o, g
            perm.extend(range(gate * 512 + G * 128, gate * 512 + (G + 1) * 128))
    return np.array(perm)


def _prepare(inputs):
    motion_state = np.asarray(inputs["motion_state"], np.float32)
    robot_state = np.asarray(inputs["robot_state"], np.float32)
    action = np.asarray(inputs["action"], np.float32)
    osc = np.asarray(inputs["osc"], np.float32)
    history = np.asarray(inputs["history"], np.float32)
    history_osc = np.asarray(inputs["history_osc"], np.float32)

    B, T = action.shape[0], action.shape[1]
    assert B == NCORES * BL
    TT = 2 * T - 1

    perm = _gate_perm()
    Wk_p = np.ascontiguousarray(np.asarray(inputs["Wk"], np.float32)[:, perm])
    Wrk_p = np.ascontiguousarray(np.asarray(inputs["Wrk"], np.float32)[:, perm])
    bl_p = np.ascontiguousarray(
        np.asarray(inputs["bl"], np.float32)[perm].reshape(1, 2048))
    Wo = np.asarray(inputs["Wo"], np.float32)  # [512, 1]
    Wo_t = np.ascontiguousarray(Wo.reshape(4, 128).T)  # [128, 4]
    bo = np.asarray(inputs["bo"], np.float32)
    bo_t = np.full((64, 1), float(bo[0]), np.float32)

    # streams: warmup over last T-1 history frames, then action/osc
    seq1 = np.concatenate([history[:, 1:], action], axis=1)          # [B, TT, 128]
    seqo = np.concatenate([history_osc[:, 1:, 0:128], osc[:, :, 0:128]], axis=1)
    seq1_tm = np.swapaxes(seq1, 0, 1)                                # [TT, B, 128]
    seqo_tm = np.swapaxes(seqo, 0, 1)

    shared = {
        "Wm": np.asarray(inputs["Wm"], np.float32),
        "bm": np.asarray(inputs["bm"], np.float32).reshape(256, 1),
        "Wr": np.asarray(inputs["Wr"], np.float32),
        "br": np.asarray(inputs["br"], np.float32).reshape(256, 1),
        "Wc": np.asarray(inputs["Wc"], np.float32),
        "bc": np.asarray(inputs["bc"], np.float32).reshape(512, 1),
        "Wor": np.asarray(inputs["Wor"], np.float32),
        "bor": np.asarray(inputs["bor"], np.float32).reshape(256, 1),
        "Woi": np.asarray(inputs["Woi"], np.float32),
        "boi": np.asarray(inputs["boi"], np.float32).reshape(256, 1),
        "Wk": Wk_p, "Wrk": Wrk_p, "bl": bl_p, "Wo": Wo_t, "bo": bo_t,
    }

    in_maps = []
    for c in range(NCORES):
        sl = slice(c * BL, (c + 1) * BL)
        m = dict(shared)
        m["mot"] = np.ascontiguousarray(motion_state[sl])
        m["rob"] = np.ascontiguousarray(robot_state[sl])
        m["s1"] = np.ascontiguousarray(seq1_tm[:, sl]).reshape(TT * BL, 128)
        m["so"] = np.ascontiguousarray(seqo_tm[:, sl]).reshape(TT * BL, 128)
        in_maps.append(m)

    return in_maps, T


def kernel(**inputs):
    in_maps, T = _prepare(inputs)
    if T not in _prog_cache:
        _prog_cache[T] = _build(T)
    nc = _prog_cache[T]

    res = run_bass_kernel_spmd(nc, in_maps, core_ids=list(range(NCORES)))
    out = np.concatenate([res.results[c]["out"] for c in range(NCORES)], axis=0)
    return out.astype(np.float32)
